# revision 3
# baseline (speedup 1.0000x reference)
"""LoRA LayerNorm Trainium2 kernel (8-core data-parallel, raw Bass).

out = x_hat * scale + shift, where
  x_hat    = (x - mean) * rsqrt(var + eps)        (LayerNorm over last dim)
  scale[i] = sum_r A_s[i,r] * B_s[r,i] * 2.0      (low-rank diagonal)
  shift[i] = sum_r A_h[i,r] * B_h[r,i] * 2.0

Sharding: x [2,4096,8192] -> 8192 rows, 1024 rows per core. LoRA params
replicated; each core computes scale/shift redundantly on device.

The output is computed and stored in float16 (tolerance is 2e-2; this
pipeline lands ~1e-3) which halves the store-side HBM traffic. The host
converts back to f32.

Per-core engine split, tuned so each engine stays at or under the DMA
cadence (~15.4 us per [128, 8192] tile = 4 MB load + 2 MB f16 store over
16 queues):
  SP  : x tile loads + y stores (HWDGE)
  DVE : sum(x) via tensor_reduce (f32, ~8.7us); small chain
        var+eps = ssq/N - (mean^2 - eps) -> reciprocal; f16 tensor_tensor
        mul/add with scale_bc/shift_bc on cols [0:HI) (2x 16-bit mode)
  ACT : sumsq via Square activation with accum_out (~7.1us);
        rstd = Sqrt(rinv); nmr = -mean*rstd; xh = Identity(x*rstd + nmr)
        f32 -> f16 (~7.1us)
  GP  : setup (LoRA diagonals to f16 broadcast tiles via DRAM bounce);
        f16 tensor_tensor mul/add on cols [HI:N) (software engine)
  PE  : unused
"""

import numpy as np
from contextlib import ExitStack

import concourse.bass as bass
from concourse import mybir
from concourse.bass_utils import run_bass_kernel_spmd

F32 = mybir.dt.float32
F16 = mybir.dt.float16

# Problem geometry (hardcoded; see module docstring)
B_DIM, S_DIM, N = 2, 4096, 8192
RANK = 4
SCALING = 2.0  # alpha / rank = 8 / 4
EPS = 1e-5
NCORES = 8
ROWS = B_DIM * S_DIM // NCORES  # 1024 rows per core
P = 128
NTILES = ROWS // P              # 8
HI = 5888                       # DVE handles cols [0:HI), GpSimd [HI:N)


def build_nc() -> bass.Bass:
    nc = bass.Bass()

    x = nc.declare_dram_parameter("x_shard", [ROWS, N], F32, isOutput=False)
    sa = nc.declare_dram_parameter("lora_scale_A", [N, RANK], F32, isOutput=False)
    sb = nc.declare_dram_parameter("lora_scale_B", [RANK, N], F32, isOutput=False)
    ha = nc.declare_dram_parameter("lora_shift_A", [N, RANK], F32, isOutput=False)
    hb = nc.declare_dram_parameter("lora_shift_B", [RANK, N], F32, isOutput=False)
    y = nc.declare_dram_parameter("y_shard", [ROWS, N], F16, isOutput=True)

    scale_vec = nc.dram_tensor("scale_vec", [N], F16)
    shift_vec = nc.dram_tensor("shift_vec", [N], F16)

    with ExitStack() as ctx:
        ec = ctx.enter_context
        # big tiles
        xb = [ec(nc.sbuf_tensor(f"xb{i}", [P, N], F32)) for i in range(2)]
        xh = [ec(nc.sbuf_tensor(f"xh{i}", [P, N], F16)) for i in range(2)]
        junk = ec(nc.sbuf_tensor("junk", [P, N], F16))
        scale_bc = ec(nc.sbuf_tensor("scale_bc", [P, N], F16))
        shift_bc = ec(nc.sbuf_tensor("shift_bc", [P, N], F16))
        # setup scratch
        a_t = ec(nc.sbuf_tensor("a_t", [P, (N // P) * RANK], F32))  # [128, 256]
        b_t = ec(nc.sbuf_tensor("b_t", [P, RANK * (N // P)], F32))  # [128, 256]
        prod = ec(nc.sbuf_tensor("prod", [P, (N // P) * RANK], F32))
        s_small = ec(nc.sbuf_tensor("s_small", [P, N // P], F32))   # [128, 64]
        t_small = ec(nc.sbuf_tensor("t_small", [P, N // P], F32))
        sv16 = ec(nc.sbuf_tensor("sv16", [P, N // P], F16))
        tv16 = ec(nc.sbuf_tensor("tv16", [P, N // P], F16))
        # per-tile stats (all [P,1])
        red = ec(nc.sbuf_tensor("red", [P, 1], F32))
        msqe = ec(nc.sbuf_tensor("msqe", [P, 1], F32))
        veps = ec(nc.sbuf_tensor("veps", [P, 1], F32))
        negmean = [ec(nc.sbuf_tensor(f"negmean{i}", [P, 1], F32)) for i in range(2)]
        ssq = [ec(nc.sbuf_tensor(f"ssq{i}", [P, 1], F32)) for i in range(2)]
        rinv = [ec(nc.sbuf_tensor(f"rinv{i}", [P, 1], F32)) for i in range(2)]
        rstd = [ec(nc.sbuf_tensor(f"rstd{i}", [P, 1], F32)) for i in range(2)]
        nmr = [ec(nc.sbuf_tensor(f"nmr{i}", [P, 1], F32)) for i in range(2)]
        eps_t = ec(nc.sbuf_tensor("eps_t", [P, 1], F32))

        sems = {}
        for s in ("load0", "load1", "store0", "store1", "sdma", "dset",
                  "gset", "ssq", "rinv", "xh", "tt", "gt", "const"):
            sems[s] = ec(nc.semaphore(s))
        loadS = [sems["load0"], sems["load1"]]
        storeS = [sems["store0"], sems["store1"]]

        C = N // P  # 64
        MUL = mybir.AluOpType.mult
        SUB = mybir.AluOpType.subtract

        with nc.Block() as block:

            @block.sync
            def _(sp):
                sp.dma_start(
                    out=xb[0][:], in_=x[0:P, :]
                ).then_inc(loadS[0], 16)
                sp.dma_start(
                    out=xb[1][:], in_=x[P:2 * P, :]
                ).then_inc(loadS[1], 16)
                for t in range(NTILES):
                    sp.wait_ge(sems["tt"], t + 1)
                    sp.wait_ge(sems["gt"], t + 1)
                    sp.dma_start(
                        out=y[t * P:(t + 1) * P, :], in_=xh[t % 2][:]
                    ).then_inc(storeS[t % 2], 16)
                    u = t + 2
                    if u < NTILES:
                        # xb[u%2] free once ACT finished xh of tile u-2
                        sp.wait_ge(sems["xh"], u - 1)
                        sp.dma_start(
                            out=xb[u % 2][:], in_=x[u * P:(u + 1) * P, :]
                        ).then_inc(loadS[u % 2], 16)

            @block.gpsimd
            def _(gp):
                # setup: load scale pair (A as [p,(c r)], B as [p,(r c)])
                gp.dma_start(
                    out=a_t[:],
                    in_=sa[:, :].rearrange("(p c) r -> p (c r)", p=P),
                ).then_inc(sems["sdma"], 16)
                gp.dma_start(
                    out=b_t[:].rearrange("p (r c) -> p r c", r=RANK),
                    in_=sb[:, :].rearrange("r (p c) -> p r c", p=P),
                ).then_inc(sems["sdma"], 16)
                gp.wait_ge(sems["dset"], 1)
                gp.dma_start(
                    out=scale_vec[:].rearrange("(p c) -> p c", p=P),
                    in_=sv16[:],
                ).then_inc(sems["gset"], 16)
                # reuse a_t/b_t for the shift pair
                gp.dma_start(
                    out=a_t[:],
                    in_=ha[:, :].rearrange("(p c) r -> p (c r)", p=P),
                ).then_inc(sems["sdma"], 16)
                gp.dma_start(
                    out=b_t[:].rearrange("p (r c) -> p r c", r=RANK),
                    in_=hb[:, :].rearrange("r (p c) -> p r c", p=P),
                ).then_inc(sems["sdma"], 16)
                gp.wait_ge(sems["dset"], 2)
                gp.dma_start(
                    out=shift_vec[:].rearrange("(p c) -> p c", p=P),
                    in_=tv16[:],
                ).then_inc(sems["gset"], 16)
                # both DRAM vectors written before reading them back
                gp.wait_ge(sems["gset"], 32)
                # broadcast along partitions (stride-0 DRAM read)
                for vec, bc in ((scale_vec, scale_bc), (shift_vec, shift_bc)):
                    vv = vec[:]
                    gp.dma_start(
                        out=bc[:],
                        in_=bass.AP(tensor=vv.tensor, offset=vv.offset,
                                    ap=[[0, P]] + list(vv.ap)),
                    ).then_inc(sems["gset"], 16)
                gp.wait_ge(sems["gset"], 64)
                # affine on cols [HI:N) for every tile
                for t in range(NTILES):
                    gp.wait_ge(sems["xh"], t + 1)
                    xt = xh[t % 2]
                    gp.tensor_mul(
                        xt[:, HI:N], xt[:, HI:N], scale_bc[:, HI:N]
                    )
                    gp.drain()
                    gp.tensor_add(
                        xt[:, HI:N], xt[:, HI:N], shift_bc[:, HI:N]
                    ).then_inc(sems["gt"], 1)

            @block.vector
            def _(v):
                v.memset(eps_t[:], EPS).then_inc(sems["const"], 1)
                # low-rank diagonals: diag = sum_r A[:,r]*B[r,:] * SCALING
                for (small, v16, k) in ((s_small, sv16, 1), (t_small, tv16, 2)):
                    v.wait_ge(sems["sdma"], 32 * k)
                    v.tensor_mul(
                        prod[:].rearrange("p (c r) -> p c r", c=C),
                        a_t[:].rearrange("p (c r) -> p c r", c=C),
                        b_t[:].rearrange("p (r c) -> p c r", r=RANK),
                    )
                    v.drain()
                    v.tensor_reduce(
                        out=small[:].rearrange("p (c u) -> p c u", u=1),
                        in_=prod[:].rearrange("p (c r) -> p c r", c=C),
                        axis=mybir.AxisListType.X,
                        op=mybir.AluOpType.add,
                    )
                    v.drain()
                    v.tensor_scalar_mul(small[:], small[:], SCALING)
                    v.drain()
                    v.tensor_copy(v16[:], small[:]).then_inc(sems["dset"], 1)

                def tt_pair(u):
                    # xh[u][:, 0:HI] = xh * scale_bc + shift_bc (f16, 2x)
                    if u == 0:
                        v.wait_ge(sems["gset"], 64)
                    v.wait_ge(sems["xh"], u + 1)
                    xt = xh[u % 2]
                    v.tensor_mul(xt[:, 0:HI], xt[:, 0:HI], scale_bc[:, 0:HI])
                    v.drain()
                    v.tensor_add(
                        xt[:, 0:HI], xt[:, 0:HI], shift_bc[:, 0:HI]
                    ).then_inc(sems["tt"], 1)

                for t in range(NTILES):
                    v.wait_ge(loadS[t % 2], 16 * (t // 2 + 1))
                    v.tensor_reduce(
                        out=red[:].rearrange("p (c u) -> p c u", u=1),
                        in_=xb[t % 2][:].rearrange("p (c f) -> p c f", c=1),
                        axis=mybir.AxisListType.X,
                        op=mybir.AluOpType.add,
                    )
                    v.drain()
                    v.tensor_scalar_mul(negmean[t % 2][:], red[:], -1.0 / N)
                    v.drain()
                    # msqe = mean^2 - eps
                    v.scalar_tensor_tensor(
                        out=msqe[:],
                        in0=negmean[t % 2][:],
                        scalar=negmean[t % 2][:, 0:1],
                        in1=eps_t[:],
                        op0=MUL,
                        op1=SUB,
                    )
                    v.drain()
                    v.wait_ge(sems["ssq"], t + 1)
                    # veps = ssq/N - (mean^2 - eps) = var + eps
                    v.scalar_tensor_tensor(
                        out=veps[:],
                        in0=ssq[t % 2][:],
                        scalar=1.0 / N,
                        in1=msqe[:],
                        op0=MUL,
                        op1=SUB,
                    )
                    v.drain()
                    v.reciprocal(rinv[t % 2][:], veps[:]).then_inc(
                        sems["rinv"], 1
                    )
                    if t >= 1:
                        tt_pair(t - 1)
                tt_pair(NTILES - 1)

            @block.scalar
            def _(sc):
                sc.wait_ge(sems["const"], 1)  # eps
                for t in range(NTILES):
                    sc.wait_ge(loadS[t % 2], 16 * (t // 2 + 1))
                    # sumsq via Square activation accumulate
                    sc.activation(
                        out=junk[:],
                        in_=xb[t % 2][:],
                        func=mybir.ActivationFunctionType.Square,
                        bias=0.0,
                        scale=1.0,
                        accum_out=ssq[t % 2][:],
                    ).then_inc(sems["ssq"], 1)
                    sc.wait_ge(sems["rinv"], t + 1)
                    sc.activation(
                        out=rstd[t % 2][:],
                        in_=rinv[t % 2][:],
                        func=mybir.ActivationFunctionType.Sqrt,
                        bias=0.0,
                        scale=1.0,
                    )
                    sc.drain()
                    # nmr = -mean * rstd
                    sc.activation(
                        out=nmr[t % 2][:],
                        in_=negmean[t % 2][:],
                        func=mybir.ActivationFunctionType.Copy,
                        bias=0.0,
                        scale=rstd[t % 2][:],
                    )
                    sc.drain()
                    if t >= 2:
                        # xh[t%2] free once store of tile t-2 done
                        sc.wait_ge(storeS[t % 2], 16 * (t // 2))
                    sc.activation(
                        out=xh[t % 2][:],
                        in_=xb[t % 2][:],
                        func=mybir.ActivationFunctionType.Identity,
                        bias=nmr[t % 2][:],
                        scale=rstd[t % 2][:],
                    ).then_inc(sems["xh"], 1)

    return nc


def kernel(x, lora_scale_A, lora_scale_B, lora_shift_A, lora_shift_B):
    x = np.ascontiguousarray(np.asarray(x, dtype=np.float32).reshape(-1, N))
    args = {
        "lora_scale_A": np.ascontiguousarray(lora_scale_A, dtype=np.float32),
        "lora_scale_B": np.ascontiguousarray(lora_scale_B, dtype=np.float32),
        "lora_shift_A": np.ascontiguousarray(lora_shift_A, dtype=np.float32),
        "lora_shift_B": np.ascontiguousarray(lora_shift_B, dtype=np.float32),
    }
    in_maps = [
        {"x_shard": x[i * ROWS:(i + 1) * ROWS], **args} for i in range(NCORES)
    ]
    nc = build_nc()
    res = run_bass_kernel_spmd(nc, in_maps, core_ids=list(range(NCORES)))
    out = np.concatenate(
        [np.asarray(res.results[i]["y_shard"]) for i in range(NCORES)], axis=0
    ).astype(np.float32)
    return out.reshape(B_DIM, S_DIM, N)


if __name__ == "__main__":
    import reference

    inputs = {k: np.asarray(v) for k, v in reference.setup_inputs().items()}
    expected = np.asarray(reference.reference(**inputs))
    actual = kernel(**inputs)
    err = np.abs(actual - expected)
    denom = np.abs(expected).max()
    print("max abs err:", err.max(), "rel:", err.max() / denom)


# revision 8
# speedup vs baseline: 1.3553x; 1.3553x over previous
"""LoRA LayerNorm Trainium2 kernel (8-core data-parallel, raw Bass).

out = x_hat * scale + shift, where
  x_hat    = (x - mean) * rsqrt(var + eps)        (LayerNorm over last dim)
  scale[i] = sum_r A_s[i,r] * B_s[r,i] * 2.0      (low-rank diagonal)
  shift[i] = sum_r A_h[i,r] * B_h[r,i] * 2.0

Sharding: x [2,4096,8192] -> 8192 rows, 1024 rows per core. LoRA params
replicated; each core computes scale/shift redundantly on device.

The output is computed and stored in float16 (tolerance is 2e-2; this
pipeline lands ~1e-3) which halves the store-side HBM traffic. The host
converts back to f32.

Per-core engine split, tuned so each engine stays at or under the DMA
cadence (~15.4 us per [128, 8192] tile = 4 MB load + 2 MB f16 store over
16 queues):
  SP  : x tile loads + y stores (HWDGE)
  DVE : sum(x) via tensor_reduce (f32, ~8.7us); small chain
        var+eps = ssq/N - (mean^2 - eps) -> reciprocal; f16 tensor_tensor
        mul/add with scale_bc/shift_bc on cols [0:HI) (2x 16-bit mode)
  ACT : sumsq via Square activation with accum_out (~7.1us);
        rstd = Sqrt(rinv); nmr = -mean*rstd; xh = Identity(x*rstd + nmr)
        f32 -> f16 (~7.1us)
  GP  : setup (LoRA diagonals to f16 broadcast tiles via DRAM bounce);
        f16 tensor_tensor mul/add on cols [HI:N) (software engine)
  PE  : unused
"""

import numpy as np
from contextlib import ExitStack

import concourse.bass as bass
from concourse import mybir
from concourse.bass_utils import run_bass_kernel_spmd

F32 = mybir.dt.float32
F16 = mybir.dt.float16

# Problem geometry (hardcoded; see module docstring)
B_DIM, S_DIM, N = 2, 4096, 8192
RANK = 4
SCALING = 2.0  # alpha / rank = 8 / 4
EPS = 1e-5
NCORES = 8
ROWS = B_DIM * S_DIM // NCORES  # 1024 rows per core
P = 128
NTILES = ROWS // P              # 8
HI = 5888                       # DVE handles cols [0:HI), GpSimd [HI:N)


def build_nc() -> bass.Bass:
    nc = bass.Bass()

    x = nc.declare_dram_parameter("x_shard", [ROWS, N], F32, isOutput=False)
    sa = nc.declare_dram_parameter("lora_scale_A", [N, RANK], F32, isOutput=False)
    sb = nc.declare_dram_parameter("lora_scale_B", [RANK, N], F32, isOutput=False)
    ha = nc.declare_dram_parameter("lora_shift_A", [N, RANK], F32, isOutput=False)
    hb = nc.declare_dram_parameter("lora_shift_B", [RANK, N], F32, isOutput=False)
    y = nc.declare_dram_parameter("y_shard", [ROWS, N], F16, isOutput=True)

    scale_vec = nc.dram_tensor("scale_vec", [N], F16)
    shift_vec = nc.dram_tensor("shift_vec", [N], F16)

    with ExitStack() as ctx:
        ec = ctx.enter_context
        # big tiles (x triple-buffered so loads run ahead of the chain)
        XB = 3
        xb = [ec(nc.sbuf_tensor(f"xb{i}", [P, N], F32)) for i in range(XB)]
        xh = [ec(nc.sbuf_tensor(f"xh{i}", [P, N], F16)) for i in range(2)]
        junk = ec(nc.sbuf_tensor("junk", [P, N], F16))
        scale_bc = ec(nc.sbuf_tensor("scale_bc", [P, N], F16))
        shift_bc = ec(nc.sbuf_tensor("shift_bc", [P, N], F16))
        # setup scratch
        a_t = ec(nc.sbuf_tensor("a_t", [P, (N // P) * RANK], F32))  # [128, 256]
        b_t = ec(nc.sbuf_tensor("b_t", [P, RANK * (N // P)], F32))  # [128, 256]
        prod = ec(nc.sbuf_tensor("prod", [P, (N // P) * RANK], F32))
        s_small = ec(nc.sbuf_tensor("s_small", [P, N // P], F32))   # [128, 64]
        t_small = ec(nc.sbuf_tensor("t_small", [P, N // P], F32))
        sv16 = ec(nc.sbuf_tensor("sv16", [P, N // P], F16))
        tv16 = ec(nc.sbuf_tensor("tv16", [P, N // P], F16))
        # per-tile stats (all [P,1])
        red = ec(nc.sbuf_tensor("red", [P, 1], F32))
        msqe = ec(nc.sbuf_tensor("msqe", [P, 1], F32))
        veps = ec(nc.sbuf_tensor("veps", [P, 1], F32))
        negmean = [ec(nc.sbuf_tensor(f"negmean{i}", [P, 1], F32)) for i in range(2)]
        ssq = [ec(nc.sbuf_tensor(f"ssq{i}", [P, 1], F32)) for i in range(2)]
        rinv = [ec(nc.sbuf_tensor(f"rinv{i}", [P, 1], F32)) for i in range(2)]
        rstd = [ec(nc.sbuf_tensor(f"rstd{i}", [P, 1], F32)) for i in range(2)]
        nmr = [ec(nc.sbuf_tensor(f"nmr{i}", [P, 1], F32)) for i in range(2)]
        eps_t = ec(nc.sbuf_tensor("eps_t", [P, 1], F32))

        sems = {}
        for s in ("load0", "load1", "load2", "store0", "store1", "sdma",
                  "dset", "gset", "ssq", "rinv", "xh", "tt", "gt", "const"):
            sems[s] = ec(nc.semaphore(s))
        loadS = [sems["load0"], sems["load1"], sems["load2"]]
        storeS = [sems["store0"], sems["store1"]]

        C = N // P  # 64
        MUL = mybir.AluOpType.mult
        SUB = mybir.AluOpType.subtract

        with nc.Block() as block:

            @block.sync
            def _(sp):
                # loads only: run as far ahead as the xb ring allows
                for t in range(NTILES):
                    if t >= XB:
                        # xb[t%XB] free once ACT finished xh of tile t-XB
                        sp.wait_ge(sems["xh"], t - XB + 1)
                    sp.dma_start(
                        out=xb[t % XB][:], in_=x[t * P:(t + 1) * P, :]
                    ).then_inc(loadS[t % XB], 16)

            @block.gpsimd
            def _(gp):
                # setup: load scale pair (A as [p,(c r)], B as [p,(r c)])
                gp.dma_start(
                    out=a_t[:],
                    in_=sa[:, :].rearrange("(p c) r -> p (c r)", p=P),
                ).then_inc(sems["sdma"], 16)
                gp.dma_start(
                    out=b_t[:].rearrange("p (r c) -> p r c", r=RANK),
                    in_=sb[:, :].rearrange("r (p c) -> p r c", p=P),
                ).then_inc(sems["sdma"], 16)
                gp.wait_ge(sems["dset"], 1)
                gp.dma_start(
                    out=scale_vec[:].rearrange("(p c) -> p c", p=P),
                    in_=sv16[:],
                ).then_inc(sems["gset"], 16)
                # reuse a_t/b_t for the shift pair
                gp.dma_start(
                    out=a_t[:],
                    in_=ha[:, :].rearrange("(p c) r -> p (c r)", p=P),
                ).then_inc(sems["sdma"], 16)
                gp.dma_start(
                    out=b_t[:].rearrange("p (r c) -> p r c", r=RANK),
                    in_=hb[:, :].rearrange("r (p c) -> p r c", p=P),
                ).then_inc(sems["sdma"], 16)
                gp.wait_ge(sems["dset"], 2)
                gp.dma_start(
                    out=shift_vec[:].rearrange("(p c) -> p c", p=P),
                    in_=tv16[:],
                ).then_inc(sems["gset"], 16)
                # both DRAM vectors written before reading them back
                gp.wait_ge(sems["gset"], 32)
                # broadcast along partitions (stride-0 DRAM read)
                for vec, bc in ((scale_vec, scale_bc), (shift_vec, shift_bc)):
                    vv = vec[:]
                    gp.dma_start(
                        out=bc[:],
                        in_=bass.AP(tensor=vv.tensor, offset=vv.offset,
                                    ap=[[0, P]] + list(vv.ap)),
                    ).then_inc(sems["gset"], 16)
                gp.wait_ge(sems["gset"], 64)
                # affine on cols [HI:N) for every tile, then issue the store
                for t in range(NTILES):
                    gp.wait_ge(sems["xh"], t + 1)
                    xt = xh[t % 2]
                    gp.tensor_mul(
                        xt[:, HI:N], xt[:, HI:N], scale_bc[:, HI:N]
                    )
                    gp.drain()
                    gp.tensor_add(
                        xt[:, HI:N], xt[:, HI:N], shift_bc[:, HI:N]
                    ).then_inc(sems["gt"], 1)
                    gp.drain()
                    gp.wait_ge(sems["tt"], t + 1)
                    gp.dma_start(
                        out=y[t * P:(t + 1) * P, :], in_=xt[:]
                    ).then_inc(storeS[t % 2], 16)

            @block.vector
            def _(v):
                v.memset(eps_t[:], EPS).then_inc(sems["const"], 1)
                # low-rank diagonals: diag = sum_r A[:,r]*B[r,:] * SCALING
                for (small, v16, k) in ((s_small, sv16, 1), (t_small, tv16, 2)):
                    v.wait_ge(sems["sdma"], 32 * k)
                    v.tensor_mul(
                        prod[:].rearrange("p (c r) -> p c r", c=C),
                        a_t[:].rearrange("p (c r) -> p c r", c=C),
                        b_t[:].rearrange("p (r c) -> p c r", r=RANK),
                    )
                    v.drain()
                    v.tensor_reduce(
                        out=small[:].rearrange("p (c u) -> p c u", u=1),
                        in_=prod[:].rearrange("p (c r) -> p c r", c=C),
                        axis=mybir.AxisListType.X,
                        op=mybir.AluOpType.add,
                    )
                    v.drain()
                    v.tensor_scalar_mul(small[:], small[:], SCALING)
                    v.drain()
                    v.tensor_copy(v16[:], small[:]).then_inc(sems["dset"], 1)

                def tt_pair(u):
                    # xh[u][:, 0:HI] = xh * scale_bc + shift_bc (f16, 2x)
                    if u == 0:
                        v.wait_ge(sems["gset"], 64)
                    v.wait_ge(sems["xh"], u + 1)
                    xt = xh[u % 2]
                    v.tensor_mul(xt[:, 0:HI], xt[:, 0:HI], scale_bc[:, 0:HI])
                    v.drain()
                    v.tensor_add(
                        xt[:, 0:HI], xt[:, 0:HI], shift_bc[:, 0:HI]
                    ).then_inc(sems["tt"], 1)

                for t in range(NTILES):
                    v.wait_ge(loadS[t % XB], 16 * (t // XB + 1))
                    v.tensor_reduce(
                        out=red[:].rearrange("p (c u) -> p c u", u=1),
                        in_=xb[t % XB][:].rearrange("p (c f) -> p c f", c=1),
                        axis=mybir.AxisListType.X,
                        op=mybir.AluOpType.add,
                    )
                    v.drain()
                    v.tensor_scalar_mul(negmean[t % 2][:], red[:], -1.0 / N)
                    v.drain()
                    # msqe = mean^2 - eps
                    v.scalar_tensor_tensor(
                        out=msqe[:],
                        in0=negmean[t % 2][:],
                        scalar=negmean[t % 2][:, 0:1],
                        in1=eps_t[:],
                        op0=MUL,
                        op1=SUB,
                    )
                    v.drain()
                    v.wait_ge(sems["ssq"], t + 1)
                    # veps = ssq/N - (mean^2 - eps) = var + eps
                    v.scalar_tensor_tensor(
                        out=veps[:],
                        in0=ssq[t % 2][:],
                        scalar=1.0 / N,
                        in1=msqe[:],
                        op0=MUL,
                        op1=SUB,
                    )
                    v.drain()
                    v.reciprocal(rinv[t % 2][:], veps[:]).then_inc(
                        sems["rinv"], 1
                    )
                    if t >= 1:
                        tt_pair(t - 1)
                tt_pair(NTILES - 1)

            @block.scalar
            def _(sc):
                sc.wait_ge(sems["const"], 1)  # eps
                for t in range(NTILES):
                    sc.wait_ge(loadS[t % XB], 16 * (t // XB + 1))
                    # sumsq via Square activation accumulate
                    sc.activation(
                        out=junk[:],
                        in_=xb[t % XB][:],
                        func=mybir.ActivationFunctionType.Square,
                        bias=0.0,
                        scale=1.0,
                        accum_out=ssq[t % 2][:],
                    ).then_inc(sems["ssq"], 1)
                    sc.wait_ge(sems["rinv"], t + 1)
                    sc.activation(
                        out=rstd[t % 2][:],
                        in_=rinv[t % 2][:],
                        func=mybir.ActivationFunctionType.Sqrt,
                        bias=0.0,
                        scale=1.0,
                    )
                    sc.drain()
                    # nmr = -mean * rstd
                    sc.activation(
                        out=nmr[t % 2][:],
                        in_=negmean[t % 2][:],
                        func=mybir.ActivationFunctionType.Copy,
                        bias=0.0,
                        scale=rstd[t % 2][:],
                    )
                    sc.drain()
                    if t >= 2:
                        # xh[t%2] free once store of tile t-2 done
                        sc.wait_ge(storeS[t % 2], 16 * (t // 2))
                    sc.activation(
                        out=xh[t % 2][:],
                        in_=xb[t % XB][:],
                        func=mybir.ActivationFunctionType.Identity,
                        bias=nmr[t % 2][:],
                        scale=rstd[t % 2][:],
                    ).then_inc(sems["xh"], 1)

    return nc


def kernel(x, lora_scale_A, lora_scale_B, lora_shift_A, lora_shift_B):
    x = np.ascontiguousarray(np.asarray(x, dtype=np.float32).reshape(-1, N))
    args = {
        "lora_scale_A": np.ascontiguousarray(lora_scale_A, dtype=np.float32),
        "lora_scale_B": np.ascontiguousarray(lora_scale_B, dtype=np.float32),
        "lora_shift_A": np.ascontiguousarray(lora_shift_A, dtype=np.float32),
        "lora_shift_B": np.ascontiguousarray(lora_shift_B, dtype=np.float32),
    }
    in_maps = [
        {"x_shard": x[i * ROWS:(i + 1) * ROWS], **args} for i in range(NCORES)
    ]
    nc = build_nc()
    res = run_bass_kernel_spmd(nc, in_maps, core_ids=list(range(NCORES)))
    out = np.concatenate(
        [np.asarray(res.results[i]["y_shard"]) for i in range(NCORES)], axis=0
    ).astype(np.float32)
    return out.reshape(B_DIM, S_DIM, N)


if __name__ == "__main__":
    import reference

    inputs = {k: np.asarray(v) for k, v in reference.setup_inputs().items()}
    expected = np.asarray(reference.reference(**inputs))
    actual = kernel(**inputs)
    err = np.abs(actual - expected)
    denom = np.abs(expected).max()
    print("max abs err:", err.max(), "rel:", err.max() / denom)


# revision 10
# speedup vs baseline: 1.4290x; 1.0544x over previous
"""LoRA LayerNorm Trainium2 kernel (8-core data-parallel, raw Bass).

out = x_hat * scale + shift, where
  x_hat    = (x - mean) * rsqrt(var + eps)        (LayerNorm over last dim)
  scale[i] = sum_r A_s[i,r] * B_s[r,i] * 2.0      (low-rank diagonal)
  shift[i] = sum_r A_h[i,r] * B_h[r,i] * 2.0

Sharding: x [2,4096,8192] -> 8192 rows, 1024 rows per core. LoRA params
replicated; each core computes scale/shift redundantly on device.

The output is computed and stored in float16 (tolerance is 2e-2; this
pipeline lands ~1e-3) which halves the store-side HBM traffic. The host
converts back to f32.

Per-core engine split, tuned so each engine stays at or under the DMA
cadence (~15.4 us per [128, 8192] tile = 4 MB load + 2 MB f16 store over
16 queues):
  SP  : x tile loads + y stores (HWDGE)
  DVE : sum(x) via tensor_reduce (f32, ~8.7us); small chain
        var+eps = ssq/N - (mean^2 - eps) -> reciprocal; f16 tensor_tensor
        mul/add with scale_bc/shift_bc on cols [0:HI) (2x 16-bit mode)
  ACT : sumsq via Square activation with accum_out (~7.1us);
        rstd = Sqrt(rinv); nmr = -mean*rstd; xh = Identity(x*rstd + nmr)
        f32 -> f16 (~7.1us)
  GP  : setup (LoRA diagonals to f16 broadcast tiles via DRAM bounce);
        f16 tensor_tensor mul/add on cols [HI:N) (software engine)
  PE  : unused
"""

import numpy as np
from contextlib import ExitStack

import concourse.bass as bass
from concourse import mybir
from concourse.bass_utils import run_bass_kernel_spmd

F32 = mybir.dt.float32
F16 = mybir.dt.float16

# Problem geometry (hardcoded; see module docstring)
B_DIM, S_DIM, N = 2, 4096, 8192
RANK = 4
SCALING = 2.0  # alpha / rank = 8 / 4
EPS = 1e-5
NCORES = 8
ROWS = B_DIM * S_DIM // NCORES  # 1024 rows per core
P = 128
NTILES = ROWS // P              # 8
HI = 5888                       # DVE handles cols [0:HI), GpSimd [HI:N)


def build_nc() -> bass.Bass:
    nc = bass.Bass()

    x = nc.declare_dram_parameter("x_shard", [ROWS, N], F32, isOutput=False)
    sa = nc.declare_dram_parameter("lora_scale_A", [N, RANK], F32, isOutput=False)
    sb = nc.declare_dram_parameter("lora_scale_B", [RANK, N], F32, isOutput=False)
    ha = nc.declare_dram_parameter("lora_shift_A", [N, RANK], F32, isOutput=False)
    hb = nc.declare_dram_parameter("lora_shift_B", [RANK, N], F32, isOutput=False)
    y = nc.declare_dram_parameter("y_shard", [ROWS, N], F16, isOutput=True)

    scale_vec = nc.dram_tensor("scale_vec", [N], F16)
    shift_vec = nc.dram_tensor("shift_vec", [N], F16)

    with ExitStack() as ctx:
        ec = ctx.enter_context
        # big tiles (x triple-buffered so loads run ahead of the chain)
        XB = 3
        xb = [ec(nc.sbuf_tensor(f"xb{i}", [P, N], F32)) for i in range(XB)]
        xh = [ec(nc.sbuf_tensor(f"xh{i}", [P, N], F16)) for i in range(2)]
        junk = ec(nc.sbuf_tensor("junk", [P, N], F16))
        scale_bc = ec(nc.sbuf_tensor("scale_bc", [P, N], F16))
        shift_bc = ec(nc.sbuf_tensor("shift_bc", [P, N], F16))
        # setup scratch
        a_t = ec(nc.sbuf_tensor("a_t", [P, (N // P) * RANK], F32))  # [128, 256]
        b_t = ec(nc.sbuf_tensor("b_t", [P, RANK * (N // P)], F32))  # [128, 256]
        prod = ec(nc.sbuf_tensor("prod", [P, (N // P) * RANK], F32))
        s_small = ec(nc.sbuf_tensor("s_small", [P, N // P], F32))   # [128, 64]
        t_small = ec(nc.sbuf_tensor("t_small", [P, N // P], F32))
        sv16 = ec(nc.sbuf_tensor("sv16", [P, N // P], F16))
        tv16 = ec(nc.sbuf_tensor("tv16", [P, N // P], F16))
        # per-tile stats (all [P,1])
        red = ec(nc.sbuf_tensor("red", [P, 1], F32))
        msqe = ec(nc.sbuf_tensor("msqe", [P, 1], F32))
        veps = ec(nc.sbuf_tensor("veps", [P, 1], F32))
        negmean = [ec(nc.sbuf_tensor(f"negmean{i}", [P, 1], F32)) for i in range(2)]
        ssq = [ec(nc.sbuf_tensor(f"ssq{i}", [P, 1], F32)) for i in range(2)]
        rinv = [ec(nc.sbuf_tensor(f"rinv{i}", [P, 1], F32)) for i in range(2)]
        rstd = [ec(nc.sbuf_tensor(f"rstd{i}", [P, 1], F32)) for i in range(2)]
        nmr = [ec(nc.sbuf_tensor(f"nmr{i}", [P, 1], F32)) for i in range(2)]
        eps_t = ec(nc.sbuf_tensor("eps_t", [P, 1], F32))

        sems = {}
        for s in ("load0", "load1", "load2", "store0", "store1", "sdma",
                  "dset", "gset", "ssq", "rinv", "xh", "tt", "gt", "const"):
            sems[s] = ec(nc.semaphore(s))
        loadS = [sems["load0"], sems["load1"], sems["load2"]]
        storeS = [sems["store0"], sems["store1"]]

        C = N // P  # 64
        MUL = mybir.AluOpType.mult
        SUB = mybir.AluOpType.subtract

        with nc.Block() as block:

            @block.sync
            def _(sp):
                # loads only: run as far ahead as the xb ring allows
                for t in range(NTILES):
                    if t >= XB:
                        # xb[t%XB] free once ACT finished xh of tile t-XB
                        sp.wait_ge(sems["xh"], t - XB + 1)
                    sp.dma_start(
                        out=xb[t % XB][:], in_=x[t * P:(t + 1) * P, :]
                    ).then_inc(loadS[t % XB], 16)

            @block.gpsimd
            def _(gp):
                # setup: load scale pair (A as [p,(c r)], B as [p,(r c)])
                gp.dma_start(
                    out=a_t[:],
                    in_=sa[:, :].rearrange("(p c) r -> p (c r)", p=P),
                ).then_inc(sems["sdma"], 16)
                gp.dma_start(
                    out=b_t[:].rearrange("p (r c) -> p r c", r=RANK),
                    in_=sb[:, :].rearrange("r (p c) -> p r c", p=P),
                ).then_inc(sems["sdma"], 16)
                gp.wait_ge(sems["dset"], 1)
                gp.dma_start(
                    out=scale_vec[:].rearrange("(p c) -> p c", p=P),
                    in_=sv16[:],
                ).then_inc(sems["gset"], 16)
                # reuse a_t/b_t for the shift pair
                gp.dma_start(
                    out=a_t[:],
                    in_=ha[:, :].rearrange("(p c) r -> p (c r)", p=P),
                ).then_inc(sems["sdma"], 16)
                gp.dma_start(
                    out=b_t[:].rearrange("p (r c) -> p r c", r=RANK),
                    in_=hb[:, :].rearrange("r (p c) -> p r c", p=P),
                ).then_inc(sems["sdma"], 16)
                gp.wait_ge(sems["dset"], 2)
                gp.dma_start(
                    out=shift_vec[:].rearrange("(p c) -> p c", p=P),
                    in_=tv16[:],
                ).then_inc(sems["gset"], 16)
                # both DRAM vectors written before reading them back
                gp.wait_ge(sems["gset"], 32)
                # broadcast along partitions (stride-0 DRAM read)
                for vec, bc in ((scale_vec, scale_bc), (shift_vec, shift_bc)):
                    vv = vec[:]
                    gp.dma_start(
                        out=bc[:],
                        in_=bass.AP(tensor=vv.tensor, offset=vv.offset,
                                    ap=[[0, P]] + list(vv.ap)),
                    ).then_inc(sems["gset"], 16)
                # stores: issue once DVE finished the affine of tile t.
                # (No gpsimd tensor ops here: Pool tensor ops serialize
                # against concurrent DVE ops on this hardware.)
                for t in range(NTILES):
                    gp.wait_ge(sems["tt"], t + 1)
                    gp.dma_start(
                        out=y[t * P:(t + 1) * P, :], in_=xh[t % 2][:]
                    ).then_inc(storeS[t % 2], 16)

            @block.vector
            def _(v):
                v.memset(eps_t[:], EPS).then_inc(sems["const"], 1)
                # low-rank diagonals: diag = sum_r A[:,r]*B[r,:] * SCALING
                for (small, v16, k) in ((s_small, sv16, 1), (t_small, tv16, 2)):
                    v.wait_ge(sems["sdma"], 32 * k)
                    v.tensor_mul(
                        prod[:].rearrange("p (c r) -> p c r", c=C),
                        a_t[:].rearrange("p (c r) -> p c r", c=C),
                        b_t[:].rearrange("p (r c) -> p c r", r=RANK),
                    )
                    v.drain()
                    v.tensor_reduce(
                        out=small[:].rearrange("p (c u) -> p c u", u=1),
                        in_=prod[:].rearrange("p (c r) -> p c r", c=C),
                        axis=mybir.AxisListType.X,
                        op=mybir.AluOpType.add,
                    )
                    v.drain()
                    v.tensor_scalar_mul(small[:], small[:], SCALING)
                    v.drain()
                    v.tensor_copy(v16[:], small[:]).then_inc(sems["dset"], 1)

                def tt_pair(u):
                    # xh[u] = xh * scale_bc + shift_bc (f16, 2x mode)
                    if u == 0:
                        v.wait_ge(sems["gset"], 64)
                    v.wait_ge(sems["xh"], u + 1)
                    xt = xh[u % 2]
                    v.tensor_mul(xt[:], xt[:], scale_bc[:])
                    v.drain()
                    v.tensor_add(
                        xt[:], xt[:], shift_bc[:]
                    ).then_inc(sems["tt"], 1)

                for t in range(NTILES):
                    v.wait_ge(loadS[t % XB], 16 * (t // XB + 1))
                    v.tensor_reduce(
                        out=red[:].rearrange("p (c u) -> p c u", u=1),
                        in_=xb[t % XB][:].rearrange("p (c f) -> p c f", c=1),
                        axis=mybir.AxisListType.X,
                        op=mybir.AluOpType.add,
                    )
                    v.drain()
                    v.tensor_scalar_mul(negmean[t % 2][:], red[:], -1.0 / N)
                    v.drain()
                    # msqe = mean^2 - eps
                    v.scalar_tensor_tensor(
                        out=msqe[:],
                        in0=negmean[t % 2][:],
                        scalar=negmean[t % 2][:, 0:1],
                        in1=eps_t[:],
                        op0=MUL,
                        op1=SUB,
                    )
                    v.drain()
                    v.wait_ge(sems["ssq"], t + 1)
                    # veps = ssq/N - (mean^2 - eps) = var + eps
                    v.scalar_tensor_tensor(
                        out=veps[:],
                        in0=ssq[t % 2][:],
                        scalar=1.0 / N,
                        in1=msqe[:],
                        op0=MUL,
                        op1=SUB,
                    )
                    v.drain()
                    v.reciprocal(rinv[t % 2][:], veps[:]).then_inc(
                        sems["rinv"], 1
                    )
                    if t >= 1:
                        tt_pair(t - 1)
                tt_pair(NTILES - 1)

            @block.scalar
            def _(sc):
                sc.wait_ge(sems["const"], 1)  # eps
                for t in range(NTILES):
                    sc.wait_ge(loadS[t % XB], 16 * (t // XB + 1))
                    # sumsq via Square activation accumulate
                    sc.activation(
                        out=junk[:],
                        in_=xb[t % XB][:],
                        func=mybir.ActivationFunctionType.Square,
                        bias=0.0,
                        scale=1.0,
                        accum_out=ssq[t % 2][:],
                    ).then_inc(sems["ssq"], 1)
                    sc.wait_ge(sems["rinv"], t + 1)
                    sc.activation(
                        out=rstd[t % 2][:],
                        in_=rinv[t % 2][:],
                        func=mybir.ActivationFunctionType.Sqrt,
                        bias=0.0,
                        scale=1.0,
                    )
                    sc.drain()
                    # nmr = -mean * rstd
                    sc.activation(
                        out=nmr[t % 2][:],
                        in_=negmean[t % 2][:],
                        func=mybir.ActivationFunctionType.Copy,
                        bias=0.0,
                        scale=rstd[t % 2][:],
                    )
                    sc.drain()
                    if t >= 2:
                        # xh[t%2] free once store of tile t-2 done
                        sc.wait_ge(storeS[t % 2], 16 * (t // 2))
                    sc.activation(
                        out=xh[t % 2][:],
                        in_=xb[t % XB][:],
                        func=mybir.ActivationFunctionType.Identity,
                        bias=nmr[t % 2][:],
                        scale=rstd[t % 2][:],
                    ).then_inc(sems["xh"], 1)

    return nc


def kernel(x, lora_scale_A, lora_scale_B, lora_shift_A, lora_shift_B):
    x = np.ascontiguousarray(np.asarray(x, dtype=np.float32).reshape(-1, N))
    args = {
        "lora_scale_A": np.ascontiguousarray(lora_scale_A, dtype=np.float32),
        "lora_scale_B": np.ascontiguousarray(lora_scale_B, dtype=np.float32),
        "lora_shift_A": np.ascontiguousarray(lora_shift_A, dtype=np.float32),
        "lora_shift_B": np.ascontiguousarray(lora_shift_B, dtype=np.float32),
    }
    in_maps = [
        {"x_shard": x[i * ROWS:(i + 1) * ROWS], **args} for i in range(NCORES)
    ]
    nc = build_nc()
    res = run_bass_kernel_spmd(nc, in_maps, core_ids=list(range(NCORES)))
    out = np.concatenate(
        [np.asarray(res.results[i]["y_shard"]) for i in range(NCORES)], axis=0
    ).astype(np.float32)
    return out.reshape(B_DIM, S_DIM, N)


if __name__ == "__main__":
    import reference

    inputs = {k: np.asarray(v) for k, v in reference.setup_inputs().items()}
    expected = np.asarray(reference.reference(**inputs))
    actual = kernel(**inputs)
    err = np.abs(actual - expected)
    denom = np.abs(expected).max()
    print("max abs err:", err.max(), "rel:", err.max() / denom)


# revision 11
# speedup vs baseline: 1.5063x; 1.0541x over previous
"""LoRA LayerNorm Trainium2 kernel (8-core data-parallel, raw Bass).

out = x_hat * scale + shift, where
  x_hat    = (x - mean) * rsqrt(var + eps)        (LayerNorm over last dim)
  scale[i] = sum_r A_s[i,r] * B_s[r,i] * 2.0      (low-rank diagonal)
  shift[i] = sum_r A_h[i,r] * B_h[r,i] * 2.0

Sharding: x [2,4096,8192] -> 8192 rows, 1024 rows per core. LoRA params
replicated; each core computes scale/shift redundantly on device.

The output is computed and stored in float16 (tolerance is 2e-2; this
pipeline lands ~1e-3) which halves the store-side HBM traffic. The host
converts back to f32.

Per-core engine split, tuned so each engine stays at or under the DMA
cadence (~15.4 us per [128, 8192] tile = 4 MB load + 2 MB f16 store over
16 queues):
  SP  : x tile loads + y stores (HWDGE)
  DVE : sum(x) via tensor_reduce (f32, ~8.7us); small chain
        var+eps = ssq/N - (mean^2 - eps) -> reciprocal; f16 tensor_tensor
        mul/add with scale_bc/shift_bc on cols [0:HI) (2x 16-bit mode)
  ACT : sumsq via Square activation with accum_out (~7.1us);
        rstd = Sqrt(rinv); nmr = -mean*rstd; xh = Identity(x*rstd + nmr)
        f32 -> f16 (~7.1us)
  GP  : setup (LoRA diagonals to f16 broadcast tiles via DRAM bounce);
        f16 tensor_tensor mul/add on cols [HI:N) (software engine)
  PE  : unused
"""

import numpy as np
from contextlib import ExitStack

import concourse.bass as bass
from concourse import mybir
from concourse.bass_utils import run_bass_kernel_spmd

F32 = mybir.dt.float32
F16 = mybir.dt.float16

# Problem geometry (hardcoded; see module docstring)
B_DIM, S_DIM, N = 2, 4096, 8192
RANK = 4
SCALING = 2.0  # alpha / rank = 8 / 4
EPS = 1e-5
NCORES = 8
ROWS = B_DIM * S_DIM // NCORES  # 1024 rows per core
P = 128
NTILES = ROWS // P              # 8
HI = 5888                       # DVE handles cols [0:HI), GpSimd [HI:N)


def build_nc() -> bass.Bass:
    nc = bass.Bass()

    x = nc.declare_dram_parameter("x_shard", [ROWS, N], F32, isOutput=False)
    sa = nc.declare_dram_parameter("lora_scale_A", [N, RANK], F32, isOutput=False)
    sb = nc.declare_dram_parameter("lora_scale_B", [RANK, N], F32, isOutput=False)
    ha = nc.declare_dram_parameter("lora_shift_A", [N, RANK], F32, isOutput=False)
    hb = nc.declare_dram_parameter("lora_shift_B", [RANK, N], F32, isOutput=False)
    y = nc.declare_dram_parameter("y_shard", [ROWS, N], F16, isOutput=True)

    scale_vec = nc.dram_tensor("scale_vec", [N], F16)
    shift_vec = nc.dram_tensor("shift_vec", [N], F16)

    with ExitStack() as ctx:
        ec = ctx.enter_context
        # big tiles (x triple-buffered so loads run ahead of the chain)
        XB = 3
        xb = [ec(nc.sbuf_tensor(f"xb{i}", [P, N], F32)) for i in range(XB)]
        XH = 3
        xh = [ec(nc.sbuf_tensor(f"xh{i}", [P, N], F16)) for i in range(XH)]
        junk = ec(nc.sbuf_tensor("junk", [P, N], F16))
        scale_bc = ec(nc.sbuf_tensor("scale_bc", [P, N], F16))
        shift_bc = ec(nc.sbuf_tensor("shift_bc", [P, N], F16))
        # setup scratch
        a_t = ec(nc.sbuf_tensor("a_t", [P, (N // P) * RANK], F32))  # [128, 256]
        b_t = ec(nc.sbuf_tensor("b_t", [P, RANK * (N // P)], F32))  # [128, 256]
        prod = ec(nc.sbuf_tensor("prod", [P, (N // P) * RANK], F32))
        s_small = ec(nc.sbuf_tensor("s_small", [P, N // P], F32))   # [128, 64]
        t_small = ec(nc.sbuf_tensor("t_small", [P, N // P], F32))
        sv16 = ec(nc.sbuf_tensor("sv16", [P, N // P], F16))
        tv16 = ec(nc.sbuf_tensor("tv16", [P, N // P], F16))
        # per-tile stats (all [P,1])
        red = ec(nc.sbuf_tensor("red", [P, 1], F32))
        msqe = ec(nc.sbuf_tensor("msqe", [P, 1], F32))
        veps = ec(nc.sbuf_tensor("veps", [P, 1], F32))
        negmean = [ec(nc.sbuf_tensor(f"negmean{i}", [P, 1], F32)) for i in range(2)]
        ssq = [ec(nc.sbuf_tensor(f"ssq{i}", [P, 1], F32)) for i in range(2)]
        rinv = [ec(nc.sbuf_tensor(f"rinv{i}", [P, 1], F32)) for i in range(2)]
        rstd = [ec(nc.sbuf_tensor(f"rstd{i}", [P, 1], F32)) for i in range(2)]
        nmr = [ec(nc.sbuf_tensor(f"nmr{i}", [P, 1], F32)) for i in range(2)]
        eps_t = ec(nc.sbuf_tensor("eps_t", [P, 1], F32))

        sems = {}
        for s in ("load0", "load1", "load2", "store0", "store1", "store2",
                  "sdma", "dset", "gset", "ssq", "rinv", "xh", "tt", "gt",
                  "const"):
            sems[s] = ec(nc.semaphore(s))
        loadS = [sems["load0"], sems["load1"], sems["load2"]]
        storeS = [sems["store0"], sems["store1"], sems["store2"]]

        C = N // P  # 64
        MUL = mybir.AluOpType.mult
        SUB = mybir.AluOpType.subtract

        with nc.Block() as block:

            @block.sync
            def _(sp):
                # loads only: run as far ahead as the xb ring allows
                for t in range(NTILES):
                    if t >= XB:
                        # xb[t%XB] free once ACT finished xh of tile t-XB
                        sp.wait_ge(sems["xh"], t - XB + 1)
                    sp.dma_start(
                        out=xb[t % XB][:], in_=x[t * P:(t + 1) * P, :]
                    ).then_inc(loadS[t % XB], 16)

            @block.gpsimd
            def _(gp):
                # setup: load scale pair (A as [p,(c r)], B as [p,(r c)])
                gp.dma_start(
                    out=a_t[:],
                    in_=sa[:, :].rearrange("(p c) r -> p (c r)", p=P),
                ).then_inc(sems["sdma"], 16)
                gp.dma_start(
                    out=b_t[:].rearrange("p (r c) -> p r c", r=RANK),
                    in_=sb[:, :].rearrange("r (p c) -> p r c", p=P),
                ).then_inc(sems["sdma"], 16)
                gp.wait_ge(sems["dset"], 1)
                gp.dma_start(
                    out=scale_vec[:].rearrange("(p c) -> p c", p=P),
                    in_=sv16[:],
                ).then_inc(sems["gset"], 16)
                # reuse a_t/b_t for the shift pair
                gp.dma_start(
                    out=a_t[:],
                    in_=ha[:, :].rearrange("(p c) r -> p (c r)", p=P),
                ).then_inc(sems["sdma"], 16)
                gp.dma_start(
                    out=b_t[:].rearrange("p (r c) -> p r c", r=RANK),
                    in_=hb[:, :].rearrange("r (p c) -> p r c", p=P),
                ).then_inc(sems["sdma"], 16)
                gp.wait_ge(sems["dset"], 2)
                gp.dma_start(
                    out=shift_vec[:].rearrange("(p c) -> p c", p=P),
                    in_=tv16[:],
                ).then_inc(sems["gset"], 16)
                # both DRAM vectors written before reading them back
                gp.wait_ge(sems["gset"], 32)
                # broadcast along partitions (stride-0 DRAM read)
                for vec, bc in ((scale_vec, scale_bc), (shift_vec, shift_bc)):
                    vv = vec[:]
                    gp.dma_start(
                        out=bc[:],
                        in_=bass.AP(tensor=vv.tensor, offset=vv.offset,
                                    ap=[[0, P]] + list(vv.ap)),
                    ).then_inc(sems["gset"], 16)
                # stores: issue once DVE finished the affine of tile t.
                # (No gpsimd tensor ops here: Pool tensor ops serialize
                # against concurrent DVE ops on this hardware.)
                for t in range(NTILES):
                    gp.wait_ge(sems["tt"], t + 1)
                    gp.dma_start(
                        out=y[t * P:(t + 1) * P, :], in_=xh[t % XH][:]
                    ).then_inc(storeS[t % XH], 16)

            @block.vector
            def _(v):
                v.memset(eps_t[:], EPS).then_inc(sems["const"], 1)
                # low-rank diagonals: diag = sum_r A[:,r]*B[r,:] * SCALING
                for (small, v16, k) in ((s_small, sv16, 1), (t_small, tv16, 2)):
                    v.wait_ge(sems["sdma"], 32 * k)
                    v.tensor_mul(
                        prod[:].rearrange("p (c r) -> p c r", c=C),
                        a_t[:].rearrange("p (c r) -> p c r", c=C),
                        b_t[:].rearrange("p (r c) -> p c r", r=RANK),
                    )
                    v.drain()
                    v.tensor_reduce(
                        out=small[:].rearrange("p (c u) -> p c u", u=1),
                        in_=prod[:].rearrange("p (c r) -> p c r", c=C),
                        axis=mybir.AxisListType.X,
                        op=mybir.AluOpType.add,
                    )
                    v.drain()
                    v.tensor_scalar_mul(small[:], small[:], SCALING)
                    v.drain()
                    v.tensor_copy(v16[:], small[:]).then_inc(sems["dset"], 1)

                def tt_pair(u):
                    # xh[u] = xh * scale_bc + shift_bc (f16, 2x mode)
                    if u == 0:
                        v.wait_ge(sems["gset"], 64)
                    v.wait_ge(sems["xh"], u + 1)
                    xt = xh[u % XH]
                    v.tensor_mul(xt[:], xt[:], scale_bc[:])
                    v.drain()
                    v.tensor_add(
                        xt[:], xt[:], shift_bc[:]
                    ).then_inc(sems["tt"], 1)

                for t in range(NTILES):
                    v.wait_ge(loadS[t % XB], 16 * (t // XB + 1))
                    v.tensor_reduce(
                        out=red[:].rearrange("p (c u) -> p c u", u=1),
                        in_=xb[t % XB][:].rearrange("p (c f) -> p c f", c=1),
                        axis=mybir.AxisListType.X,
                        op=mybir.AluOpType.add,
                    )
                    v.drain()
                    v.tensor_scalar_mul(negmean[t % 2][:], red[:], -1.0 / N)
                    v.drain()
                    # msqe = mean^2 - eps
                    v.scalar_tensor_tensor(
                        out=msqe[:],
                        in0=negmean[t % 2][:],
                        scalar=negmean[t % 2][:, 0:1],
                        in1=eps_t[:],
                        op0=MUL,
                        op1=SUB,
                    )
                    v.drain()
                    v.wait_ge(sems["ssq"], t + 1)
                    # veps = ssq/N - (mean^2 - eps) = var + eps
                    v.scalar_tensor_tensor(
                        out=veps[:],
                        in0=ssq[t % 2][:],
                        scalar=1.0 / N,
                        in1=msqe[:],
                        op0=MUL,
                        op1=SUB,
                    )
                    v.drain()
                    v.reciprocal(rinv[t % 2][:], veps[:]).then_inc(
                        sems["rinv"], 1
                    )
                    if t >= 1:
                        tt_pair(t - 1)
                tt_pair(NTILES - 1)

            @block.scalar
            def _(sc):
                sc.wait_ge(sems["const"], 1)  # eps
                for t in range(NTILES):
                    sc.wait_ge(loadS[t % XB], 16 * (t // XB + 1))
                    # sumsq via Square activation accumulate
                    sc.activation(
                        out=junk[:],
                        in_=xb[t % XB][:],
                        func=mybir.ActivationFunctionType.Square,
                        bias=0.0,
                        scale=1.0,
                        accum_out=ssq[t % 2][:],
                    ).then_inc(sems["ssq"], 1)
                    sc.wait_ge(sems["rinv"], t + 1)
                    sc.activation(
                        out=rstd[t % 2][:],
                        in_=rinv[t % 2][:],
                        func=mybir.ActivationFunctionType.Sqrt,
                        bias=0.0,
                        scale=1.0,
                    )
                    sc.drain()
                    # nmr = -mean * rstd
                    sc.activation(
                        out=nmr[t % 2][:],
                        in_=negmean[t % 2][:],
                        func=mybir.ActivationFunctionType.Copy,
                        bias=0.0,
                        scale=rstd[t % 2][:],
                    )
                    sc.drain()
                    if t >= XH:
                        # xh[t%XH] free once store of tile t-XH done
                        sc.wait_ge(storeS[t % XH], 16 * (t // XH))
                    sc.activation(
                        out=xh[t % XH][:],
                        in_=xb[t % XB][:],
                        func=mybir.ActivationFunctionType.Identity,
                        bias=nmr[t % 2][:],
                        scale=rstd[t % 2][:],
                    ).then_inc(sems["xh"], 1)

    return nc


def kernel(x, lora_scale_A, lora_scale_B, lora_shift_A, lora_shift_B):
    x = np.ascontiguousarray(np.asarray(x, dtype=np.float32).reshape(-1, N))
    args = {
        "lora_scale_A": np.ascontiguousarray(lora_scale_A, dtype=np.float32),
        "lora_scale_B": np.ascontiguousarray(lora_scale_B, dtype=np.float32),
        "lora_shift_A": np.ascontiguousarray(lora_shift_A, dtype=np.float32),
        "lora_shift_B": np.ascontiguousarray(lora_shift_B, dtype=np.float32),
    }
    in_maps = [
        {"x_shard": x[i * ROWS:(i + 1) * ROWS], **args} for i in range(NCORES)
    ]
    nc = build_nc()
    res = run_bass_kernel_spmd(nc, in_maps, core_ids=list(range(NCORES)))
    out = np.concatenate(
        [np.asarray(res.results[i]["y_shard"]) for i in range(NCORES)], axis=0
    ).astype(np.float32)
    return out.reshape(B_DIM, S_DIM, N)


if __name__ == "__main__":
    import reference

    inputs = {k: np.asarray(v) for k, v in reference.setup_inputs().items()}
    expected = np.asarray(reference.reference(**inputs))
    actual = kernel(**inputs)
    err = np.abs(actual - expected)
    denom = np.abs(expected).max()
    print("max abs err:", err.max(), "rel:", err.max() / denom)


# revision 13
# speedup vs baseline: 1.5336x; 1.0181x over previous
"""LoRA LayerNorm Trainium2 kernel (8-core data-parallel, raw Bass).

out = x_hat * scale + shift, where
  x_hat    = (x - mean) * rsqrt(var + eps)        (LayerNorm over last dim)
  scale[i] = sum_r A_s[i,r] * B_s[r,i] * 2.0      (low-rank diagonal)
  shift[i] = sum_r A_h[i,r] * B_h[r,i] * 2.0

Sharding: x [2,4096,8192] -> 8192 rows, 1024 rows per core. LoRA params
replicated; each core computes scale/shift redundantly on device.

The output is computed and stored in float16 (tolerance is 2e-2; this
pipeline lands ~1e-3) which halves the store-side HBM traffic. The host
converts back to f32.

Per-core engine split, tuned so each engine stays at or under the DMA
cadence (~15.4 us per [128, 8192] tile = 4 MB load + 2 MB f16 store over
16 queues):
  SP  : x tile loads + y stores (HWDGE)
  DVE : sum(x) via tensor_reduce (f32, ~8.7us); small chain
        var+eps = ssq/N - (mean^2 - eps) -> reciprocal; f16 tensor_tensor
        mul/add with scale_bc/shift_bc on cols [0:HI) (2x 16-bit mode)
  ACT : sumsq via Square activation with accum_out (~7.1us);
        rstd = Sqrt(rinv); nmr = -mean*rstd; xh = Identity(x*rstd + nmr)
        f32 -> f16 (~7.1us)
  GP  : setup (LoRA diagonals to f16 broadcast tiles via DRAM bounce);
        f16 tensor_tensor mul/add on cols [HI:N) (software engine)
  PE  : unused
"""

import numpy as np
from contextlib import ExitStack

import concourse.bass as bass
from concourse import mybir
from concourse.bass_utils import run_bass_kernel_spmd

F32 = mybir.dt.float32
F16 = mybir.dt.float16

# Problem geometry (hardcoded; see module docstring)
B_DIM, S_DIM, N = 2, 4096, 8192
RANK = 4
SCALING = 2.0  # alpha / rank = 8 / 4
EPS = 1e-5
NCORES = 8
ROWS = B_DIM * S_DIM // NCORES  # 1024 rows per core
P = 128
NTILES = ROWS // P              # 8
HI = 5888                       # DVE handles cols [0:HI), GpSimd [HI:N)


def build_nc() -> bass.Bass:
    nc = bass.Bass()

    x = nc.declare_dram_parameter("x_shard", [ROWS, N], F32, isOutput=False)
    sa = nc.declare_dram_parameter("lora_scale_A", [N, RANK], F32, isOutput=False)
    sb = nc.declare_dram_parameter("lora_scale_B", [RANK, N], F32, isOutput=False)
    ha = nc.declare_dram_parameter("lora_shift_A", [N, RANK], F32, isOutput=False)
    hb = nc.declare_dram_parameter("lora_shift_B", [RANK, N], F32, isOutput=False)
    y = nc.declare_dram_parameter("y_shard", [ROWS, N], F16, isOutput=True)

    scale_vec = nc.dram_tensor("scale_vec", [N], F16)
    shift_vec = nc.dram_tensor("shift_vec", [N], F16)

    with ExitStack() as ctx:
        ec = ctx.enter_context
        # big tiles (x triple-buffered so loads run ahead of the chain)
        XB = 3
        xb = [ec(nc.sbuf_tensor(f"xb{i}", [P, N], F32)) for i in range(XB)]
        XH = 3
        xh = [ec(nc.sbuf_tensor(f"xh{i}", [P, N], F16)) for i in range(XH)]
        junk = ec(nc.sbuf_tensor("junk", [P, N], F16))
        scale_bc = ec(nc.sbuf_tensor("scale_bc", [P, N], F16))
        shift_bc = ec(nc.sbuf_tensor("shift_bc", [P, N], F16))
        # setup scratch
        a_t = ec(nc.sbuf_tensor("a_t", [P, (N // P) * RANK], F32))  # [128, 256]
        b_t = ec(nc.sbuf_tensor("b_t", [P, RANK * (N // P)], F32))  # [128, 256]
        a2_t = ec(nc.sbuf_tensor("a2_t", [P, (N // P) * RANK], F32))
        b2_t = ec(nc.sbuf_tensor("b2_t", [P, RANK * (N // P)], F32))
        prod = ec(nc.sbuf_tensor("prod", [P, (N // P) * RANK], F32))
        s_small = ec(nc.sbuf_tensor("s_small", [P, N // P], F32))   # [128, 64]
        t_small = ec(nc.sbuf_tensor("t_small", [P, N // P], F32))
        sv16 = ec(nc.sbuf_tensor("sv16", [P, N // P], F16))
        tv16 = ec(nc.sbuf_tensor("tv16", [P, N // P], F16))
        # per-tile stats (all [P,1])
        red = ec(nc.sbuf_tensor("red", [P, 1], F32))
        msqe = ec(nc.sbuf_tensor("msqe", [P, 1], F32))
        veps = ec(nc.sbuf_tensor("veps", [P, 1], F32))
        negmean = [ec(nc.sbuf_tensor(f"negmean{i}", [P, 1], F32)) for i in range(2)]
        ssq = [ec(nc.sbuf_tensor(f"ssq{i}", [P, 1], F32)) for i in range(2)]
        rinv = [ec(nc.sbuf_tensor(f"rinv{i}", [P, 1], F32)) for i in range(2)]
        rstd = [ec(nc.sbuf_tensor(f"rstd{i}", [P, 1], F32)) for i in range(2)]
        nmr = [ec(nc.sbuf_tensor(f"nmr{i}", [P, 1], F32)) for i in range(2)]
        eps_t = ec(nc.sbuf_tensor("eps_t", [P, 1], F32))

        sems = {}
        for s in ("load0", "load1", "load2", "store0", "store1", "store2",
                  "sdma", "dset", "gset", "ssq", "rinv", "xh", "tt", "gt",
                  "xh7a", "tt7a", "const"):
            sems[s] = ec(nc.semaphore(s))
        loadS = [sems["load0"], sems["load1"], sems["load2"]]
        storeS = [sems["store0"], sems["store1"], sems["store2"]]

        C = N // P  # 64
        MUL = mybir.AluOpType.mult
        SUB = mybir.AluOpType.subtract

        with nc.Block() as block:

            @block.sync
            def _(sp):
                # loads only: run as far ahead as the xb ring allows
                for t in range(NTILES):
                    if t >= XB:
                        # xb[t%XB] free once ACT finished xh of tile t-XB
                        sp.wait_ge(sems["xh"], t - XB + 1)
                    sp.dma_start(
                        out=xb[t % XB][:], in_=x[t * P:(t + 1) * P, :]
                    ).then_inc(loadS[t % XB], 16)

            @block.gpsimd
            def _(gp):
                # setup: all four strided LoRA loads in parallel
                gp.dma_start(
                    out=a_t[:],
                    in_=sa[:, :].rearrange("(p c) r -> p (c r)", p=P),
                ).then_inc(sems["sdma"], 16)
                gp.dma_start(
                    out=b_t[:].rearrange("p (r c) -> p r c", r=RANK),
                    in_=sb[:, :].rearrange("r (p c) -> p r c", p=P),
                ).then_inc(sems["sdma"], 16)
                gp.dma_start(
                    out=a2_t[:],
                    in_=ha[:, :].rearrange("(p c) r -> p (c r)", p=P),
                ).then_inc(sems["sdma"], 16)
                gp.dma_start(
                    out=b2_t[:].rearrange("p (r c) -> p r c", r=RANK),
                    in_=hb[:, :].rearrange("r (p c) -> p r c", p=P),
                ).then_inc(sems["sdma"], 16)
                gp.wait_ge(sems["dset"], 1)
                gp.dma_start(
                    out=scale_vec[:].rearrange("(p c) -> p c", p=P),
                    in_=sv16[:],
                ).then_inc(sems["gset"], 16)
                gp.wait_ge(sems["dset"], 2)
                gp.dma_start(
                    out=shift_vec[:].rearrange("(p c) -> p c", p=P),
                    in_=tv16[:],
                ).then_inc(sems["gset"], 16)
                # both DRAM vectors written before reading them back
                gp.wait_ge(sems["gset"], 32)
                # broadcast along partitions (stride-0 DRAM read)
                for vec, bc in ((scale_vec, scale_bc), (shift_vec, shift_bc)):
                    vv = vec[:]
                    gp.dma_start(
                        out=bc[:],
                        in_=bass.AP(tensor=vv.tensor, offset=vv.offset,
                                    ap=[[0, P]] + list(vv.ap)),
                    ).then_inc(sems["gset"], 16)
                # stores: issue once DVE finished the affine of tile t.
                # (No gpsimd tensor ops here: Pool tensor ops serialize
                # against concurrent DVE ops on this hardware.)
                for t in range(NTILES - 1):
                    gp.wait_ge(sems["tt"], t + 1)
                    gp.dma_start(
                        out=y[t * P:(t + 1) * P, :], in_=xh[t % XH][:]
                    ).then_inc(storeS[t % XH], 16)
                t = NTILES - 1
                HF = N // 2
                gp.wait_ge(sems["tt7a"], 1)
                gp.dma_start(
                    out=y[t * P:(t + 1) * P, 0:HF],
                    in_=xh[t % XH][:, 0:HF],
                ).then_inc(storeS[t % XH], 16)
                gp.wait_ge(sems["tt"], t + 1)
                gp.dma_start(
                    out=y[t * P:(t + 1) * P, HF:N],
                    in_=xh[t % XH][:, HF:N],
                ).then_inc(storeS[t % XH], 16)

            @block.vector
            def _(v):
                v.memset(eps_t[:], EPS).then_inc(sems["const"], 1)
                # low-rank diagonals: diag = sum_r A[:,r]*B[r,:] * SCALING
                for (small, v16, at, bt, k) in (
                    (s_small, sv16, a_t, b_t, 1),
                    (t_small, tv16, a2_t, b2_t, 2),
                ):
                    v.wait_ge(sems["sdma"], 32 * k)
                    v.tensor_mul(
                        prod[:].rearrange("p (c r) -> p c r", c=C),
                        at[:].rearrange("p (c r) -> p c r", c=C),
                        bt[:].rearrange("p (r c) -> p c r", r=RANK),
                    )
                    v.drain()
                    v.tensor_reduce(
                        out=small[:].rearrange("p (c u) -> p c u", u=1),
                        in_=prod[:].rearrange("p (c r) -> p c r", c=C),
                        axis=mybir.AxisListType.X,
                        op=mybir.AluOpType.add,
                    )
                    v.drain()
                    v.tensor_scalar_mul(small[:], small[:], SCALING)
                    v.drain()
                    v.tensor_copy(v16[:], small[:]).then_inc(sems["dset"], 1)

                def tt_pair(u):
                    # xh[u] = xh * scale_bc + shift_bc (f16, 2x mode)
                    if u == 0:
                        v.wait_ge(sems["gset"], 64)
                    v.wait_ge(sems["xh"], u + 1)
                    xt = xh[u % XH]
                    v.tensor_mul(xt[:], xt[:], scale_bc[:])
                    v.drain()
                    v.tensor_add(
                        xt[:], xt[:], shift_bc[:]
                    ).then_inc(sems["tt"], 1)

                for t in range(NTILES):
                    v.wait_ge(loadS[t % XB], 16 * (t // XB + 1))
                    v.tensor_reduce(
                        out=red[:].rearrange("p (c u) -> p c u", u=1),
                        in_=xb[t % XB][:].rearrange("p (c f) -> p c f", c=1),
                        axis=mybir.AxisListType.X,
                        op=mybir.AluOpType.add,
                    )
                    v.drain()
                    v.tensor_scalar_mul(negmean[t % 2][:], red[:], -1.0 / N)
                    v.drain()
                    # msqe = mean^2 - eps
                    v.scalar_tensor_tensor(
                        out=msqe[:],
                        in0=negmean[t % 2][:],
                        scalar=negmean[t % 2][:, 0:1],
                        in1=eps_t[:],
                        op0=MUL,
                        op1=SUB,
                    )
                    v.drain()
                    v.wait_ge(sems["ssq"], t + 1)
                    # veps = ssq/N - (mean^2 - eps) = var + eps
                    v.scalar_tensor_tensor(
                        out=veps[:],
                        in0=ssq[t % 2][:],
                        scalar=1.0 / N,
                        in1=msqe[:],
                        op0=MUL,
                        op1=SUB,
                    )
                    v.drain()
                    v.reciprocal(rinv[t % 2][:], veps[:]).then_inc(
                        sems["rinv"], 1
                    )
                    if t >= 1:
                        tt_pair(t - 1)
                # final tile in column halves to shorten the drain
                u = NTILES - 1
                xt = xh[u % XH]
                HF = N // 2
                v.wait_ge(sems["xh7a"], 1)
                v.tensor_mul(xt[:, 0:HF], xt[:, 0:HF], scale_bc[:, 0:HF])
                v.drain()
                v.tensor_add(
                    xt[:, 0:HF], xt[:, 0:HF], shift_bc[:, 0:HF]
                ).then_inc(sems["tt7a"], 1)
                v.wait_ge(sems["xh"], u + 1)
                v.tensor_mul(xt[:, HF:N], xt[:, HF:N], scale_bc[:, HF:N])
                v.drain()
                v.tensor_add(
                    xt[:, HF:N], xt[:, HF:N], shift_bc[:, HF:N]
                ).then_inc(sems["tt"], 1)

            @block.scalar
            def _(sc):
                sc.wait_ge(sems["const"], 1)  # eps
                # preload activation function tables off the critical path
                sc.activation(
                    out=junk[:, 0:1], in_=eps_t[:],
                    func=mybir.ActivationFunctionType.Square,
                    bias=0.0, scale=1.0,
                )
                sc.activation(
                    out=junk[:, 0:1], in_=eps_t[:],
                    func=mybir.ActivationFunctionType.Identity,
                    bias=0.0, scale=1.0,
                )
                sc.drain()
                for t in range(NTILES):
                    sc.wait_ge(loadS[t % XB], 16 * (t // XB + 1))
                    # sumsq via Square activation accumulate
                    sc.activation(
                        out=junk[:],
                        in_=xb[t % XB][:],
                        func=mybir.ActivationFunctionType.Square,
                        bias=0.0,
                        scale=1.0,
                        accum_out=ssq[t % 2][:],
                    ).then_inc(sems["ssq"], 1)
                    sc.wait_ge(sems["rinv"], t + 1)
                    sc.activation(
                        out=rstd[t % 2][:],
                        in_=rinv[t % 2][:],
                        func=mybir.ActivationFunctionType.Sqrt,
                        bias=0.0,
                        scale=1.0,
                    )
                    sc.drain()
                    # nmr = -mean * rstd
                    sc.activation(
                        out=nmr[t % 2][:],
                        in_=negmean[t % 2][:],
                        func=mybir.ActivationFunctionType.Copy,
                        bias=0.0,
                        scale=rstd[t % 2][:],
                    )
                    sc.drain()
                    if t >= XH:
                        # xh[t%XH] free once store of tile t-XH done
                        sc.wait_ge(storeS[t % XH], 16 * (t // XH))
                    if t < NTILES - 1:
                        sc.activation(
                            out=xh[t % XH][:],
                            in_=xb[t % XB][:],
                            func=mybir.ActivationFunctionType.Identity,
                            bias=nmr[t % 2][:],
                            scale=rstd[t % 2][:],
                        ).then_inc(sems["xh"], 1)
                    else:
                        HF = N // 2
                        sc.activation(
                            out=xh[t % XH][:, 0:HF],
                            in_=xb[t % XB][:, 0:HF],
                            func=mybir.ActivationFunctionType.Identity,
                            bias=nmr[t % 2][:],
                            scale=rstd[t % 2][:],
                        ).then_inc(sems["xh7a"], 1)
                        sc.activation(
                            out=xh[t % XH][:, HF:N],
                            in_=xb[t % XB][:, HF:N],
                            func=mybir.ActivationFunctionType.Identity,
                            bias=nmr[t % 2][:],
                            scale=rstd[t % 2][:],
                        ).then_inc(sems["xh"], 1)

    return nc


def kernel(x, lora_scale_A, lora_scale_B, lora_shift_A, lora_shift_B):
    x = np.ascontiguousarray(np.asarray(x, dtype=np.float32).reshape(-1, N))
    args = {
        "lora_scale_A": np.ascontiguousarray(lora_scale_A, dtype=np.float32),
        "lora_scale_B": np.ascontiguousarray(lora_scale_B, dtype=np.float32),
        "lora_shift_A": np.ascontiguousarray(lora_shift_A, dtype=np.float32),
        "lora_shift_B": np.ascontiguousarray(lora_shift_B, dtype=np.float32),
    }
    in_maps = [
        {"x_shard": x[i * ROWS:(i + 1) * ROWS], **args} for i in range(NCORES)
    ]
    nc = build_nc()
    res = run_bass_kernel_spmd(nc, in_maps, core_ids=list(range(NCORES)))
    out = np.concatenate(
        [np.asarray(res.results[i]["y_shard"]) for i in range(NCORES)], axis=0
    ).astype(np.float32)
    return out.reshape(B_DIM, S_DIM, N)


if __name__ == "__main__":
    import reference

    inputs = {k: np.asarray(v) for k, v in reference.setup_inputs().items()}
    expected = np.asarray(reference.reference(**inputs))
    actual = kernel(**inputs)
    err = np.abs(actual - expected)
    denom = np.abs(expected).max()
    print("max abs err:", err.max(), "rel:", err.max() / denom)


# revision 14
# speedup vs baseline: 1.6090x; 1.0491x over previous
"""LoRA LayerNorm Trainium2 kernel (8-core data-parallel, raw Bass).

out = x_hat * scale + shift, where
  x_hat    = (x - mean) * rsqrt(var + eps)        (LayerNorm over last dim)
  scale[i] = sum_r A_s[i,r] * B_s[r,i] * 2.0      (low-rank diagonal)
  shift[i] = sum_r A_h[i,r] * B_h[r,i] * 2.0

Sharding: x [2,4096,8192] -> 8192 rows, 1024 rows per core. LoRA params
replicated; each core computes scale/shift redundantly on device.

The output is computed and stored in float16 (tolerance is 2e-2; this
pipeline lands ~1e-3) which halves the store-side HBM traffic. The host
converts back to f32.

Engine split, each engine at or under the DMA cadence (~15.4 us per
[128, 8192] tile = 4 MB f32 load + 2 MB f16 store over 16 queues):
  SP  : x tile loads (HWDGE), triple-buffered
  ACT : stats engine -- sum(x) via Identity+accum_out (~7.1us) and
        sumsq via Square+accum_out (~7.1us); rstd = Sqrt(rinv)
  DVE : transform engine -- small chain (negmean, mean^2-eps,
        var+eps = ssq/N - msqe, reciprocal, nmr = negmean*rstd), then
        xh = (x*rstd + nmr) via dual-scalar tensor_scalar f32->f16 at
        2x rate (~4.5us), then xh = xh*scale_bc + shift_bc as two f16
        tensor_tensor ops at 2x 16-bit rate (~8.8us)
  GP  : setup (LoRA diagonals -> f16 broadcast tiles via DRAM bounce);
        y stores (SWDGE), issued as soon as DVE finishes a tile
  PE  : unused
The last tile runs xh/affine/store in column halves to shorten the
pipeline drain.
"""

import numpy as np
from contextlib import ExitStack

import concourse.bass as bass
from concourse import mybir
from concourse.bass_utils import run_bass_kernel_spmd

F32 = mybir.dt.float32
F16 = mybir.dt.float16

# Problem geometry (hardcoded; see module docstring)
B_DIM, S_DIM, N = 2, 4096, 8192
RANK = 4
SCALING = 2.0  # alpha / rank = 8 / 4
EPS = 1e-5
NCORES = 8
ROWS = B_DIM * S_DIM // NCORES  # 1024 rows per core
P = 128
NTILES = ROWS // P              # 8
HF = N // 2


def build_nc() -> bass.Bass:
    nc = bass.Bass()

    x = nc.declare_dram_parameter("x_shard", [ROWS, N], F32, isOutput=False)
    sa = nc.declare_dram_parameter("lora_scale_A", [N, RANK], F32, isOutput=False)
    sb = nc.declare_dram_parameter("lora_scale_B", [RANK, N], F32, isOutput=False)
    ha = nc.declare_dram_parameter("lora_shift_A", [N, RANK], F32, isOutput=False)
    hb = nc.declare_dram_parameter("lora_shift_B", [RANK, N], F32, isOutput=False)
    y = nc.declare_dram_parameter("y_shard", [ROWS, N], F16, isOutput=True)

    scale_vec = nc.dram_tensor("scale_vec", [N], F16)
    shift_vec = nc.dram_tensor("shift_vec", [N], F16)

    with ExitStack() as ctx:
        ec = ctx.enter_context
        XB = 3   # x ring
        XH = 3   # xh ring
        xb = [ec(nc.sbuf_tensor(f"xb{i}", [P, N], F32)) for i in range(XB)]
        xh = [ec(nc.sbuf_tensor(f"xh{i}", [P, N], F16)) for i in range(XH)]
        junk = ec(nc.sbuf_tensor("junk", [P, N], F16))
        scale_bc = ec(nc.sbuf_tensor("scale_bc", [P, N], F16))
        shift_bc = ec(nc.sbuf_tensor("shift_bc", [P, N], F16))
        # setup scratch
        C = N // P  # 64
        a_t = ec(nc.sbuf_tensor("a_t", [P, C * RANK], F32))
        b_t = ec(nc.sbuf_tensor("b_t", [P, RANK * C], F32))
        a2_t = ec(nc.sbuf_tensor("a2_t", [P, C * RANK], F32))
        b2_t = ec(nc.sbuf_tensor("b2_t", [P, RANK * C], F32))
        prod = ec(nc.sbuf_tensor("prod", [P, C * RANK], F32))
        s_small = ec(nc.sbuf_tensor("s_small", [P, C], F32))
        t_small = ec(nc.sbuf_tensor("t_small", [P, C], F32))
        sv16 = ec(nc.sbuf_tensor("sv16", [P, C], F16))
        tv16 = ec(nc.sbuf_tensor("tv16", [P, C], F16))
        # per-tile stats (all [P,1])
        msqe = ec(nc.sbuf_tensor("msqe", [P, 1], F32))
        veps = ec(nc.sbuf_tensor("veps", [P, 1], F32))
        negmean = [ec(nc.sbuf_tensor(f"negmean{i}", [P, 1], F32)) for i in range(2)]
        acc = [ec(nc.sbuf_tensor(f"acc{i}", [P, 1], F32)) for i in range(2)]
        ssq = [ec(nc.sbuf_tensor(f"ssq{i}", [P, 1], F32)) for i in range(2)]
        rinv = [ec(nc.sbuf_tensor(f"rinv{i}", [P, 1], F32)) for i in range(2)]
        rstd = [ec(nc.sbuf_tensor(f"rstd{i}", [P, 1], F32)) for i in range(2)]
        nmr = [ec(nc.sbuf_tensor(f"nmr{i}", [P, 1], F32)) for i in range(2)]
        eps_t = ec(nc.sbuf_tensor("eps_t", [P, 1], F32))

        sems = {}
        for s in ("load0", "load1", "load2", "store0", "store1", "store2",
                  "sdma", "dset", "gset", "stat", "rinv", "std", "xhv",
                  "tt", "tt7a", "const"):
            sems[s] = ec(nc.semaphore(s))
        loadS = [sems["load0"], sems["load1"], sems["load2"]]
        storeS = [sems["store0"], sems["store1"], sems["store2"]]

        MUL = mybir.AluOpType.mult
        ADD = mybir.AluOpType.add
        SUB = mybir.AluOpType.subtract
        IDENT = mybir.ActivationFunctionType.Identity
        SQUARE = mybir.ActivationFunctionType.Square
        SQRT = mybir.ActivationFunctionType.Sqrt

        with nc.Block() as block:

            @block.sync
            def _(sp):
                # loads only: run as far ahead as the xb ring allows
                for t in range(NTILES):
                    if t >= XB:
                        # xb[t%XB] free once DVE's xh pass of t-XB is done
                        sp.wait_ge(sems["xhv"], t - XB + 1)
                    sp.dma_start(
                        out=xb[t % XB][:], in_=x[t * P:(t + 1) * P, :]
                    ).then_inc(loadS[t % XB], 16)

            @block.gpsimd
            def _(gp):
                # let the first x tile win the queues, then fire the four
                # strided LoRA loads in parallel
                gp.wait_ge(loadS[0], 8)
                gp.dma_start(
                    out=a_t[:],
                    in_=sa[:, :].rearrange("(p c) r -> p (c r)", p=P),
                ).then_inc(sems["sdma"], 16)
                gp.dma_start(
                    out=b_t[:].rearrange("p (r c) -> p r c", r=RANK),
                    in_=sb[:, :].rearrange("r (p c) -> p r c", p=P),
                ).then_inc(sems["sdma"], 16)
                gp.dma_start(
                    out=a2_t[:],
                    in_=ha[:, :].rearrange("(p c) r -> p (c r)", p=P),
                ).then_inc(sems["sdma"], 16)
                gp.dma_start(
                    out=b2_t[:].rearrange("p (r c) -> p r c", r=RANK),
                    in_=hb[:, :].rearrange("r (p c) -> p r c", p=P),
                ).then_inc(sems["sdma"], 16)
                gp.wait_ge(sems["dset"], 1)
                gp.dma_start(
                    out=scale_vec[:].rearrange("(p c) -> p c", p=P),
                    in_=sv16[:],
                ).then_inc(sems["gset"], 16)
                gp.wait_ge(sems["dset"], 2)
                gp.dma_start(
                    out=shift_vec[:].rearrange("(p c) -> p c", p=P),
                    in_=tv16[:],
                ).then_inc(sems["gset"], 16)
                gp.wait_ge(sems["gset"], 32)
                # broadcast along partitions (stride-0 DRAM read)
                for vec, bc in ((scale_vec, scale_bc), (shift_vec, shift_bc)):
                    vv = vec[:]
                    gp.dma_start(
                        out=bc[:],
                        in_=bass.AP(tensor=vv.tensor, offset=vv.offset,
                                    ap=[[0, P]] + list(vv.ap)),
                    ).then_inc(sems["gset"], 16)
                # stores: issue as soon as DVE finishes the affine of tile t
                for t in range(NTILES - 1):
                    gp.wait_ge(sems["tt"], t + 1)
                    gp.dma_start(
                        out=y[t * P:(t + 1) * P, :], in_=xh[t % XH][:]
                    ).then_inc(storeS[t % XH], 16)
                t = NTILES - 1
                gp.wait_ge(sems["tt7a"], 1)
                gp.dma_start(
                    out=y[t * P:(t + 1) * P, 0:HF],
                    in_=xh[t % XH][:, 0:HF],
                ).then_inc(storeS[t % XH], 16)
                gp.wait_ge(sems["tt"], t + 1)
                gp.dma_start(
                    out=y[t * P:(t + 1) * P, HF:N],
                    in_=xh[t % XH][:, HF:N],
                ).then_inc(storeS[t % XH], 16)

            @block.vector
            def _(v):
                v.memset(eps_t[:], EPS).then_inc(sems["const"], 1)
                # low-rank diagonals: diag = sum_r A[:,r]*B[r,:] * SCALING
                for (small, v16, at, bt, k) in (
                    (s_small, sv16, a_t, b_t, 1),
                    (t_small, tv16, a2_t, b2_t, 2),
                ):
                    v.wait_ge(sems["sdma"], 32 * k)
                    v.tensor_mul(
                        prod[:].rearrange("p (c r) -> p c r", c=C),
                        at[:].rearrange("p (c r) -> p c r", c=C),
                        bt[:].rearrange("p (r c) -> p c r", r=RANK),
                    )
                    v.drain()
                    v.tensor_reduce(
                        out=small[:].rearrange("p (c u) -> p c u", u=1),
                        in_=prod[:].rearrange("p (c r) -> p c r", c=C),
                        axis=mybir.AxisListType.X,
                        op=ADD,
                    )
                    v.drain()
                    v.tensor_scalar_mul(small[:], small[:], SCALING)
                    v.drain()
                    v.tensor_copy(v16[:], small[:]).then_inc(sems["dset"], 1)

                def xh_affine(t, lo, hi, sem_name, inc_xhv):
                    # xh = (x*rstd + nmr) * scale_bc + shift_bc on [lo:hi)
                    b2, b3 = t % 2, t % XH
                    ts = v.tensor_scalar(
                        out=xh[b3][:, lo:hi], in0=xb[t % XB][:, lo:hi],
                        scalar1=rstd[b2][:], scalar2=nmr[b2][:],
                        op0=MUL, op1=ADD,
                    )
                    if inc_xhv:
                        ts.then_inc(sems["xhv"], 1)
                    v.drain()
                    v.tensor_mul(
                        xh[b3][:, lo:hi], xh[b3][:, lo:hi],
                        scale_bc[:, lo:hi],
                    )
                    v.drain()
                    v.tensor_add(
                        xh[b3][:, lo:hi], xh[b3][:, lo:hi],
                        shift_bc[:, lo:hi],
                    ).then_inc(sems[sem_name], 1)

                for t in range(NTILES):
                    b2 = t % 2
                    v.wait_ge(sems["stat"], 2 * (t + 1))
                    v.tensor_scalar_mul(negmean[b2][:], acc[b2][:], -1.0 / N)
                    v.drain()
                    # msqe = mean^2 - eps
                    v.scalar_tensor_tensor(
                        out=msqe[:], in0=negmean[b2][:],
                        scalar=negmean[b2][:, 0:1], in1=eps_t[:],
                        op0=MUL, op1=SUB,
                    )
                    v.drain()
                    # veps = ssq/N - (mean^2 - eps) = var + eps
                    v.scalar_tensor_tensor(
                        out=veps[:], in0=ssq[b2][:], scalar=1.0 / N,
                        in1=msqe[:], op0=MUL, op1=SUB,
                    )
                    v.drain()
                    v.reciprocal(rinv[b2][:], veps[:]).then_inc(
                        sems["rinv"], 1
                    )
                    v.wait_ge(sems["std"], t + 1)
                    # nmr = -mean * rstd
                    v.tensor_mul(nmr[b2][:], negmean[b2][:], rstd[b2][:])
                    v.drain()
                    if t == 0:
                        v.wait_ge(sems["gset"], 64)
                    if t >= XH:
                        # xh[t%XH] free once store of tile t-XH done
                        v.wait_ge(storeS[t % XH], 16 * (t // XH))
                    if t < NTILES - 1:
                        xh_affine(t, 0, N, "tt", True)
                    else:
                        xh_affine(t, 0, HF, "tt7a", False)
                        xh_affine(t, HF, N, "tt", True)

            @block.scalar
            def _(sc):
                sc.wait_ge(sems["const"], 1)  # eps
                # preload activation tables off the critical path
                sc.activation(out=junk[:, 0:1], in_=eps_t[:], func=SQUARE,
                              bias=0.0, scale=1.0)
                sc.activation(out=junk[:, 0:1], in_=eps_t[:], func=IDENT,
                              bias=0.0, scale=1.0)
                sc.drain()
                for t in range(NTILES):
                    b2 = t % 2
                    sc.wait_ge(loadS[t % XB], 16 * (t // XB + 1))
                    # sum(x)
                    sc.activation(
                        out=junk[:], in_=xb[t % XB][:], func=IDENT,
                        bias=0.0, scale=1.0, accum_out=acc[b2][:],
                    ).then_inc(sems["stat"], 1)
                    # sum(x^2)
                    sc.activation(
                        out=junk[:], in_=xb[t % XB][:], func=SQUARE,
                        bias=0.0, scale=1.0, accum_out=ssq[b2][:],
                    ).then_inc(sems["stat"], 1)
                    sc.wait_ge(sems["rinv"], t + 1)
                    sc.activation(
                        out=rstd[b2][:], in_=rinv[b2][:], func=SQRT,
                        bias=0.0, scale=1.0,
                    ).then_inc(sems["std"], 1)

    return nc


def kernel(x, lora_scale_A, lora_scale_B, lora_shift_A, lora_shift_B):
    x = np.ascontiguousarray(np.asarray(x, dtype=np.float32).reshape(-1, N))
    args = {
        "lora_scale_A": np.ascontiguousarray(lora_scale_A, dtype=np.float32),
        "lora_scale_B": np.ascontiguousarray(lora_scale_B, dtype=np.float32),
        "lora_shift_A": np.ascontiguousarray(lora_shift_A, dtype=np.float32),
        "lora_shift_B": np.ascontiguousarray(lora_shift_B, dtype=np.float32),
    }
    in_maps = [
        {"x_shard": x[i * ROWS:(i + 1) * ROWS], **args} for i in range(NCORES)
    ]
    nc = build_nc()
    res = run_bass_kernel_spmd(nc, in_maps, core_ids=list(range(NCORES)))
    out = np.concatenate(
        [np.asarray(res.results[i]["y_shard"]) for i in range(NCORES)], axis=0
    ).astype(np.float32)
    return out.reshape(B_DIM, S_DIM, N)


if __name__ == "__main__":
    import reference

    inputs = {k: np.asarray(v) for k, v in reference.setup_inputs().items()}
    expected = np.asarray(reference.reference(**inputs))
    actual = kernel(**inputs)
    err = np.abs(actual - expected)
    denom = np.abs(expected).max()
    print("max abs err:", err.max(), "rel:", err.max() / denom)
